# revision 46
# baseline (speedup 1.0000x reference)
"""MatchingNet forward on 8 Trainium2 NeuronCores (Bass/Tile).

Math (reference):
    s_emb = l2norm(support @ W + b)   [Ns, E]
    q_emb = l2norm(query @ W + b)     [Nq, E]
    sims  = q_emb @ s_emb.T           [Nq, Ns]
    preds = softmax(sims, axis=1) @ one_hot(labels, C)   [Nq, C]

Sharding: query rows data-parallel (1024/core); support encode sharded
(512 rows/core) with the normalized embeddings AllGathered on-chip.

Schedule (built from 8 traced HW runs; the binding constraint is the
CC stream: a runtime-internal barrier occupies it from ~21us for
22-48us, each AllGather then costs ~10us trigger delay + ~18-37us, and
the FIRST post-barrier collective also absorbs inter-core skew -- so
gathered data cannot exist before ~75-110us no matter what):
 - support encode runs FIRST, as TWO 256-row blocks. Each block's
   normalize chain ships its AllGather chunk the moment it exists, so
   the doorbells ring at ~45-52us -- before/at the barrier end -- and
   peer skew is absorbed outside the AG op. The N=256 LDWEIGHTS-bound
   penalty (~6us of PE) hides under the collective pipeline.
 - the ship chain avoids both the Tile scheduler's reordering and the
   DRAM round trip: sum-of-squares -> sqrt -> fast-reciprocal ->
   partition-broadcast via a 1-row f32 PE matmul into PSUM -> muls,
   all emitted under tc.high_priority().
 - sims consume gather chunks CHUNK-MAJOR (both query halves of chunk
   0 before chunk 1), giving AllGather 1 ~35us of PE cover.
 - collectives + gather reads live EXCLUSIVELY on the gpsimd queue;
   sync/scalar/vector streams never wait on a collective semaphore
   (a collective-gated wait parked ahead of encoder work head-of-line
   blocks that engine and stalls the PE).
 - per-block sum-of-squares ones-matmuls trail the main groups by two
   (support) or one full block (queries), so the PE never waits on the
   ACT->DVE square chain.
 - preds for query-half 0 run between the two sims halves, so their
   divide+store tails hide under remaining sims; softmax-denominator
   reciprocals use the fast custom-DVE reciprocal (~5x faster).

Device layout: embeddings are computed TRANSPOSED ([emb, n], emb on
partitions) so the chain needs no transposes; one_hot is augmented
with a ones column so the softmax denominator falls out of the preds
matmul. Cosine sims are in [-1,1]: no max subtraction needed. Matmul
inputs are fp8e4m3 (DoubleRow, fp32 PSUM): normalized embeddings
scaled by 16, W by 32. exp stays bf16 -- fp8 exp costs ~10x in max rel
err (softmax weights concentrate; quantization noise doesn't average
out). Measured max rel err ~1.8e-3 vs the 2e-2 gate.
"""

import numpy as np
import ml_dtypes

import concourse.bacc as bacc
import concourse.mybir as mybir
import concourse.tile as tile
from concourse.bass_utils import run_bass_kernel_spmd

F32 = mybir.dt.float32
BF16 = mybir.dt.bfloat16
FP8 = mybir.dt.float8e4
EMB_SCALE = 16.0
W_SCALE = 32.0
AF = mybir.ActivationFunctionType
DR = mybir.MatmulPerfMode.DoubleRow

# Full-problem config (hardcoded; the grading harness provides exactly these)
N_SUPPORT = 4096
N_QUERY = 8192
IN_DIM = 2048
EMB_DIM = 1024
N_CLS = 64
N_CORES = 8
NQ_SHARD = N_QUERY // N_CORES  # 1024 query rows per core

# AllGather chunk widths (support rows per chunk, per core). Two chunks:
# per-AG cost is fixed-dominated (~15-23us regardless of size), so more,
# smaller chunks just stretch the serial CC pipeline; with two, chunk 0's
# chunk-major sims (~35us) fully cover AllGather 1.
CWS = (256, 256)
OFFS = (0, 256)


def build_nc(NS, NQ, IN, EMB, NCLS, n_cores=N_CORES):
    """Per-core Bass program. NS is GLOBAL support count, NQ the per-core
    query count. NCLS includes the +1 ones column."""
    KCH = IN // 128    # contraction chunks for the encoder matmul
    MCH = EMB // 128   # emb chunks (partition blocks of the embT layout)
    SCH = NS // 128    # support chunks
    NS_SH = NS // n_cores
    NB_Q = NQ // 512
    G = len(CWS)
    assert NS_SH == sum(CWS) and NQ % 512 == 0 and IN % 256 == 0
    assert all(cw % 128 == 0 for cw in CWS)

    nc = bacc.Bacc()
    # host-pre-laid-out inputs (see _prep_inputs): every DMA is contiguous
    supX = nc.declare_dram_parameter("supX", [2, 128, KCH, NS_SH // 2], FP8,
                                     isOutput=False)
    qX = nc.declare_dram_parameter("qX", [NB_Q, 128, KCH, 512], FP8,
                                   isOutput=False)
    Wd = nc.declare_dram_parameter("W", [MCH, 128, KCH, 128], FP8,
                                   isOutput=False)
    bd = nc.declare_dram_parameter("b", [128, MCH], F32, isOutput=False)
    ohd = nc.declare_dram_parameter("onehot", [128, SCH, NCLS], BF16,
                                    isOutput=False)
    outd = nc.declare_dram_parameter("out", [NQ, NCLS - 1], F32, isOutput=True)

    with tile.TileContext(nc) as tc:
        with (
            tc.tile_pool(name="singles", bufs=1) as singles,
            tc.tile_pool(name="emb_pool", bufs=1) as emb_pool,
            tc.tile_pool(name="small", bufs=2) as small,
            tc.tile_pool(name="w_pool", bufs=1) as w_pool,
            tc.tile_pool(name="xin", bufs=2) as xin,
            tc.tile_pool(name="pre_pool", bufs=2) as pre_pool,
            tc.tile_pool(name="sq_pool", bufs=2) as sq_pool,
            tc.tile_pool(name="bc_pool", bufs=2) as bc_pool,
            tc.tile_pool(name="loc_pool", bufs=1) as loc_pool,
            tc.tile_pool(name="exp_pool", bufs=1) as exp_pool,
            tc.tile_pool(name="outp", bufs=2) as outp,
            tc.tile_pool(name="dscr", bufs=2, space="DRAM") as dscr,
            tc.tile_pool(name="cc_pool", bufs=1, space="DRAM") as cc_pool,
            tc.tile_pool(name="ps_mm", bufs=3, space="PSUM") as ps_mm,
            tc.tile_pool(name="ps_n2", bufs=1, space="PSUM") as ps_n2,
            tc.tile_pool(name="ps_bc", bufs=1, space="PSUM") as ps_bc,
            tc.tile_pool(name="ps_pred", bufs=2, space="PSUM") as ps_pred,
            tc.tile_pool(name="ps_pred2", bufs=1, space="PSUM") as ps_pred2,
        ):
            ones_sb = singles.tile([128, 1], BF16)
            nc.vector.memset(ones_sb, 1.0)
            ones_row = singles.tile([1, 128], F32)
            nc.vector.memset(ones_row, 1.0)
            zeros_sb = singles.tile([128, 4 * NCLS], BF16)
            nc.vector.memset(zeros_sb, 0.0)

            # ---- inputs, first-needed first, split across queues
            b_sb = singles.tile([128, MCH], F32)
            nc.scalar.dma_start(out=b_sb, in_=bd[:, :])
            W_sb = [w_pool.tile([128, KCH, 128], FP8, tag=f"w{m}",
                                name=f"w{m}") for m in range(MCH)]
            # W[0..3] on scalar: the first support block streams on
            # sync+gpsimd in parallel, and the early W tiles no longer queue
            # behind it (v8 stalled 5.5us waiting for W2/W3 mid-encode)
            for m in range(2):
                nc.scalar.dma_start(out=W_sb[m], in_=Wd[m])
            oh_sb = singles.tile([128, SCH, NCLS], BF16)
            nc.scalar.dma_start(out=oh_sb, in_=ohd[:, :, :])
            sup_xks = []
            sA = xin.tile([128, KCH, NS_SH // 2], FP8, tag="sxk", name="sxk")
            nc.sync.dma_start(out=sA[:, 0:KCH // 2, :],
                              in_=supX[0][:, 0:KCH // 2, :])
            nc.gpsimd.dma_start(out=sA[:, KCH // 2:KCH, :],
                                in_=supX[0][:, KCH // 2:KCH, :])
            sup_xks.append(sA)
            # W[2..3] ride sync right behind the first support half: the
            # scalar queue's ACT preamble delays its triggers ~8us, which
            # left the m=2/3 groups waiting ~4us for weights
            for m in range(2, 4):
                nc.sync.dma_start(out=W_sb[m], in_=Wd[m])
            sB = xin.tile([128, KCH, NS_SH // 2], FP8, tag="sxk", name="sxk")
            nc.gpsimd.dma_start(out=sB, in_=supX[1])
            sup_xks.append(sB)
            for m in range(4, MCH):
                nc.gpsimd.dma_start(out=W_sb[m], in_=Wd[m])
            q_xks = []
            for j, eng in zip(range(NB_Q), (nc.sync, nc.scalar)):
                t = xin.tile([128, KCH, 512], FP8, tag="xk", name="xk")
                eng.dma_start(out=t, in_=qX[j])
                q_xks.append(t)

            # ---- persistent result tiles
            q_nrm = emb_pool.tile([128, MCH, NQ], FP8, name="q_nrm",
                                  tag="q_nrm")
            s_loc = loc_pool.tile([128, MCH, NS_SH], FP8, name="s_loc")
            gt = [[emb_pool.tile([128, MCH, CWS[g]], FP8, name=f"gt{g}_{c}",
                                 tag=f"gt{g}_{c}")
                   for c in range(n_cores)] for g in range(G)]
            # bf16 exp: fp8 here costs ~10x in max rel err (softmax weights
            # concentrate, so quantization noise doesn't average out)
            expT = exp_pool.tile([128, SCH, NQ], BF16)

            # ---- encode block machinery; emission is hand-interleaved so
            # every engine's instruction stream is in execution-time order
            def make_block(xk, res_tile, res_off, bs):
                st = {
                    "pre": pre_pool.tile([128, MCH, bs], BF16, tag="pre",
                                         name="pre"),
                    "sq": sq_pool.tile([128, MCH, bs], BF16, tag="sq",
                                       name="sq"),
                    "n2": None,
                }

                def group(m):
                    ps = ps_mm.tile([128, bs], F32, tag="mmps", name="ps")
                    for t in range(KCH // 2):
                        nc.tensor.matmul(
                            ps,
                            lhsT=W_sb[m][:, 2 * t:2 * t + 2, :],
                            rhs=xk[:, 2 * t:2 * t + 2, :],
                            start=(t == 0), stop=(t == KCH // 2 - 1),
                            perf_mode=DR,
                        )
                    # bias add (rescaling the fp8 W) + PSUM->SBUF bf16
                    nc.scalar.activation(st["pre"][:, m, :], ps, AF.Identity,
                                         bias=b_sb[:, m:m + 1],
                                         scale=1.0 / W_SCALE)
                    nc.vector.tensor_mul(st["sq"][:, m, :], st["pre"][:, m, :],
                                         st["pre"][:, m, :])

                def ones(m):
                    # column sums of squares via ones-matmul (partition
                    # reduce); callers sequence m = 0..MCH-1 trailing the
                    # main groups so the PE never waits on the square chain
                    if m == 0:
                        st["n2"] = ps_n2.tile([1, bs], F32, tag="n2",
                                              name="n2")
                    nc.tensor.matmul(st["n2"], lhsT=ones_sb,
                                     rhs=st["sq"][:, m, :],
                                     start=(m == 0), stop=(m == MCH - 1))

                def normalize(slices, ship_cb=None):
                    # ship path (support block): the whole chain runs at
                    # high priority AND broadcasts inv across partitions via
                    # a 1-row PE matmul into PSUM instead of the DMA round
                    # trip through DRAM -- the Tile scheduler's cost-model
                    # timeline otherwise parks the muls behind later blocks'
                    # square ops, which in v3/v4 pushed the AllGather
                    # doorbell from ~45us to ~60us (and the first AG then
                    # absorbs peer skew in-op: +20us on its duration)
                    import contextlib
                    prio = (tc.high_priority() if ship_cb is not None
                            else contextlib.nullcontext())
                    with prio:
                        nrm = small.tile([1, bs], F32, tag="nrm", name="nrm")
                        nc.scalar.activation(nrm, st["n2"], AF.Sqrt,
                                             scale=1.0 / (EMB_SCALE * EMB_SCALE))
                        inv = small.tile([1, bs], F32, tag="inv", name="inv")
                        nc.vector.reciprocal_approx_fast(inv, nrm)
                        if ship_cb is not None:
                            invb = ps_bc.tile([128, bs], F32, tag="invb",
                                              name="invb")
                            nc.tensor.matmul(invb, lhsT=ones_row, rhs=inv)
                        else:
                            # partition-broadcast via DRAM (off critical
                            # path; DMA needs a DRAM source for zero
                            # partition step)
                            iscr = dscr.tile([1, bs], F32, tag="iscr",
                                             name="iscr")
                            nc.sync.dma_start(out=iscr, in_=inv)
                            invb = bc_pool.tile([128, bs], F32, tag="invb",
                                                name="invb")
                            nc.sync.dma_start(
                                out=invb, in_=iscr.partition_broadcast(128))
                        for si, (lo, hi) in enumerate(slices):
                            for m in range(MCH):
                                nc.vector.tensor_mul(
                                    res_tile[:, m, res_off + lo:res_off + hi],
                                    st["pre"][:, m, lo:hi], invb[:, lo:hi])
                            if ship_cb is not None:
                                ship_cb(si)

                return group, ones, normalize

            ag_outs = []

            def ship(g):
                # each normalized chunk AllGathers the moment it exists
                cw = CWS[g]
                ag_in = cc_pool.tile([128, MCH * cw], FP8, name=f"ag_in{g}",
                                     tag=f"ag_in{g}")
                nc.sync.dma_start(
                    out=ag_in.rearrange("p (m v) -> p m v", m=MCH),
                    in_=s_loc[:, :, OFFS[g]:OFFS[g] + cw])
                ag_out = cc_pool.tile(
                    [n_cores * 128, MCH * cw], FP8, name=f"ag_out{g}",
                    tag=f"ag_out{g}", addr_space="Shared")
                nc.gpsimd.collective_compute(
                    "AllGather", mybir.AluOpType.bypass,
                    replica_groups=[list(range(n_cores))],
                    ins=[ag_in], outs=[ag_out],
                )
                ag_outs.append(ag_out)

            # support runs as TWO 256-row blocks: chunk 0's full
            # normalize+ship chain completes ~20us earlier than with one
            # 512-row block, so the AllGather doorbell beats the CC-stream
            # barrier and peer skew is absorbed before the op (not in it).
            # The N=256 LDWEIGHTS-bound penalty (~6us PE) is free: the
            # encode tail hides under the collective pipeline anyway.
            SAg, SAones, SAnorm = make_block(sup_xks[0], s_loc, 0, 256)
            SBg, SBones, SBnorm = make_block(sup_xks[1], s_loc, 256, 256)
            Q0g, Q0ones, Q0norm = make_block(q_xks[0], q_nrm, 0, 512)
            Q1g, Q1ones, Q1norm = make_block(q_xks[1], q_nrm, 512, 512)

            for m in range(MCH):
                SAg(m)
                if m >= 2:
                    SAones(m - 2)
            SBg(0)
            SAones(MCH - 2)
            SAones(MCH - 1)
            SBg(1)
            SAnorm([(0, 256)], ship_cb=lambda si: ship(0))
            for m in range(2, MCH):
                SBg(m)
                if m >= 4:
                    SBones(m - 4)
            Q0g(0)
            for m in range(MCH - 4, MCH):
                SBones(m)
            Q0g(1)
            SBnorm([(0, 256)], ship_cb=lambda si: ship(1))
            for m in range(2, MCH):
                Q0g(m)
            Q1g(0)
            for m in range(MCH):
                Q0ones(m)
            Q1g(1)
            Q0norm([(0, 512)])
            for m in range(2, MCH):
                Q1g(m)

            # gather-read DMAs: gpsimd queue ONLY (they wait on collective
            # semaphores; nothing else may queue behind them)
            for g in range(G):
                for c in range(n_cores):
                    nc.gpsimd.dma_start(
                        out=gt[g][c],
                        in_=ag_outs[g][c * 128:(c + 1) * 128, :]
                            .rearrange("p (m v) -> p m v", m=MCH),
                    )

            # ---- sims + softmax-exp, [sup, q] layout, chunk-arrival order
            work = [(c * (NS_SH // 128) + OFFS[g] // 128 + i, gt[g][c], i)
                    for g in range(G)
                    for c in range(n_cores)
                    for i in range(CWS[g] // 128)]
            assert MCH % 2 == 0 and len(work) == SCH

            def sims_tile(sb, src, i, qh):
                qs = slice(qh * 512, (qh + 1) * 512)
                ps = ps_mm.tile([128, 512], F32, tag="mmps", name="ps")
                for t in range(MCH // 2):
                    nc.tensor.matmul(
                        ps,
                        lhsT=src[:, 2 * t:2 * t + 2, i * 128:(i + 1) * 128],
                        rhs=q_nrm[:, 2 * t:2 * t + 2, qs],
                        start=(t == 0), stop=(t == MCH // 2 - 1),
                        perf_mode=DR,
                    )
                nc.scalar.activation(expT[:, sb, qs], ps, AF.Exp,
                                     scale=1.0 / (EMB_SCALE * EMB_SCALE))

            # support-chunk index sets for split preds accumulation
            CH0 = sorted(c * (NS_SH // 128) + i for c in range(n_cores)
                         for i in range(CWS[0] // 128))
            CH1 = [sb for sb in range(SCH) if sb not in set(CH0)]
            pp_open = {}

            def preds_div(qb, pp):
                # softmax denominator is the ones column; divide
                qs = slice(qb * 128, (qb + 1) * 128)
                rec = small.tile([128, 1], F32, tag="rec", name="rec")
                nc.vector.reciprocal_approx_fast(rec, pp[:, NCLS - 1:NCLS])
                ot = outp.tile([128, NCLS - 1], F32, tag="ot", name="ot")
                nc.vector.tensor_scalar_mul(ot, pp[:, 0:NCLS - 1], rec)
                nc.sync.dma_start(out=outd[qs, :], in_=ot)

            def preds(qb):
                qs = slice(qb * 128, (qb + 1) * 128)
                pp = ps_pred.tile([128, NCLS], F32, tag="pp", name="pp")
                for sb in range(SCH):
                    nc.tensor.matmul(pp, lhsT=expT[:, sb, qs],
                                     rhs=oh_sb[:, sb, :],
                                     start=(sb == 0), stop=(sb == SCH - 1))
                preds_div(qb, pp)

            def preds_start(qb):
                # accumulate the chunk-0 half mid-sims; group stays open.
                # All four open accumulators share one PSUM bank (pools are
                # bank-granular; 4 x 65 fp32 = 1040B fits one 2KB bank).
                # start=True clears the whole BANK, so the bank is zeroed
                # ONCE by a dummy matmul against a zero rhs, and every real
                # accumulation uses start=False.
                qs = slice(qb * 128, (qb + 1) * 128)
                if "t" not in pp_open:
                    t = ps_pred2.tile([128, 4, NCLS], F32, tag="pp2",
                                      name="pp2")
                    pp_open["t"] = t
                    nc.tensor.matmul(
                        t.rearrange("p a c -> p (a c)"),
                        lhsT=W_sb[0][:, 0:1, :], rhs=zeros_sb,
                        start=True, stop=False, skip_group_check=True)
                pp = pp_open["t"][:, qb - NQ // 256, :]
                pp_open[qb] = pp
                for k, sb in enumerate(CH0):
                    nc.tensor.matmul(pp, lhsT=expT[:, sb, qs],
                                     rhs=oh_sb[:, sb, :],
                                     start=False, stop=False,
                                     skip_group_check=True)

            def preds_finish(qb):
                qs = slice(qb * 128, (qb + 1) * 128)
                pp = pp_open[qb]
                for k, sb in enumerate(CH1):
                    nc.tensor.matmul(pp, lhsT=expT[:, sb, qs],
                                     rhs=oh_sb[:, sb, :],
                                     start=False, stop=(k == len(CH1) - 1),
                                     skip_group_check=True)
                preds_div(qb, pp)

            # chunk-major sims: BOTH query halves of chunk 0 run before
            # chunk 1 is touched, so AllGather 1 gets ~35us of PE cover
            # after AllGather 0 lands (CC timing varies 2x run to run).
            # Q1's deferred ones + normalize slot in behind the first two
            # tiles (their PE cover absorbs the square-chain lag).
            ws, pos = [], 0
            for g in range(G):
                n_g = n_cores * (CWS[g] // 128)
                ws.append(work[pos:pos + n_g])
                pos += n_g
            # Q1's deferred ones + normalize go BEFORE the first sims tile:
            # sims tile 0 blocks on AllGather 0, and this ~2.5us of ready
            # work would otherwise sit parked behind it during the stall
            for m in range(MCH):
                Q1ones(m)
            Q1norm([(0, 512)])
            for wk in ws[0]:
                sims_tile(*wk, 0)
            for wk in ws[0]:
                sims_tile(*wk, 1)
            wl = ws[G - 1]
            # chunk-0 halves of the qh=1 preds accumulate mid-sims (two
            # chunk-1 tiles first cover the exp tail of chunk 0's qh=1),
            # halving the preds work left after the final sims tile
            sims_tile(*wl[0], 0)
            sims_tile(*wl[1], 0)
            for qb in range(NQ // 256, NQ // 128):
                preds_start(qb)
            for wk in wl[2:]:
                sims_tile(*wk, 0)
            # preds for query-half 0 slot in after two qh=1 tiles so their
            # divide+store tails hide under the remaining sims
            sims_tile(*wl[0], 1)
            sims_tile(*wl[1], 1)
            for qb in range(NQ // 256):
                preds(qb)
            for wk in wl[2:]:
                sims_tile(*wk, 1)
            for qb in range(NQ // 256, NQ // 128):
                preds_finish(qb)

    nc.finalize()
    return nc


_NC_CACHE = {}


def _get_nc(key):
    if key not in _NC_CACHE:
        NS, NQ, IN, EMB, NCLS = key
        _NC_CACHE[key] = build_nc(NS, NQ, IN, EMB, NCLS)
    return _NC_CACHE[key]


def _x_layout(x, kch, bs=512):
    """[NV, IN] fp32 -> [NV/bs, 128, KCH, bs] fp8 so each bs-row encoder
    block is one contiguous DMA: H[nb,p,k,v] = x[nb*bs+v, k*128+p]."""
    nv, in_dim = x.shape
    h = x.reshape(nv // bs, bs, kch, 128).transpose(0, 3, 2, 1)
    return np.ascontiguousarray(h.astype(ml_dtypes.float8_e4m3))


def _prep_inputs(support, query, W, b, support_labels, num_classes, n_cores):
    ncls = int(num_classes)
    bf = ml_dtypes.bfloat16
    support = np.asarray(support, np.float32)
    query = np.asarray(query, np.float32)
    W = np.asarray(W, np.float32)
    in_dim, emb = W.shape
    kch, mch = in_dim // 128, emb // 128
    ns = support.shape[0]
    # W[m, p, k, e] = W_SCALE * W[k*128+p, m*128+e]
    Wh = np.ascontiguousarray(
        (W * W_SCALE).reshape(kch, 128, mch, 128)
        .transpose(2, 1, 0, 3).astype(ml_dtypes.float8_e4m3))
    # b[p, m] = b[m*128+p]
    bh = np.ascontiguousarray(np.asarray(b, np.float32).reshape(mch, 128).T)
    labels = np.asarray(support_labels).astype(np.int64)
    oh = np.zeros((ns, ncls + 1), dtype=bf)
    oh[np.arange(ns), labels] = 1
    oh[:, ncls] = 1  # ones column -> softmax denominator
    # oh[p, c, h] = onehot[c*128+p, h]
    ohh = np.ascontiguousarray(
        oh.reshape(ns // 128, 128, ncls + 1).transpose(1, 0, 2))
    nq_shard = query.shape[0] // n_cores
    ns_shard = ns // n_cores
    qh_all = _x_layout(query, kch)  # [NQ/512, 128, KCH, 512]
    nbq = nq_shard // 512
    in_maps = []
    for i in range(n_cores):
        supx = _x_layout(support[i * ns_shard:(i + 1) * ns_shard], kch,
                         ns_shard // 2)  # [2, 128, KCH, NS_SH/2]
        in_maps.append({
            "supX": supx,
            "qX": np.ascontiguousarray(qh_all[i * nbq:(i + 1) * nbq]),
            "W": Wh,
            "b": bh,
            "onehot": ohh,
        })
    return in_maps


def _run(support, query, W, b, support_labels, num_classes, trace=False):
    ncls = int(num_classes)
    key = (support.shape[0], query.shape[0] // N_CORES, support.shape[1],
           W.shape[1], ncls + 1)
    nc = _get_nc(key)
    in_maps = _prep_inputs(support, query, W, b, support_labels, ncls, N_CORES)
    res = run_bass_kernel_spmd(nc, in_maps, list(range(N_CORES)), trace=trace)
    out = np.concatenate([r["out"] for r in res.results], axis=0)
    return out.astype(np.float32), res


def kernel(support, query, W, b, support_labels, num_classes):
    out, _ = _run(support, query, W, b, support_labels, num_classes, trace=False)
    return out


# revision 48
# speedup vs baseline: 1.0606x; 1.0606x over previous
"""MatchingNet forward on 8 Trainium2 NeuronCores (Bass/Tile).

Math (reference):
    s_emb = l2norm(support @ W + b)   [Ns, E]
    q_emb = l2norm(query @ W + b)     [Nq, E]
    sims  = q_emb @ s_emb.T           [Nq, Ns]
    preds = softmax(sims, axis=1) @ one_hot(labels, C)   [Nq, C]

Sharding: query rows data-parallel (1024/core); support encode sharded
(512 rows/core) with the normalized embeddings AllGathered on-chip.

Schedule (built from 8 traced HW runs; the binding constraint is the
CC stream: a runtime-internal barrier occupies it from ~21us for
22-48us, each AllGather then costs ~10us trigger delay + ~18-37us, and
the FIRST post-barrier collective also absorbs inter-core skew -- so
gathered data cannot exist before ~75-110us no matter what):
 - support encode runs FIRST, as TWO 256-row blocks. Each block's
   normalize chain ships its AllGather chunk the moment it exists, so
   the doorbells ring at ~45-52us -- before/at the barrier end -- and
   peer skew is absorbed outside the AG op. The N=256 LDWEIGHTS-bound
   penalty (~6us of PE) hides under the collective pipeline.
 - the ship chain avoids both the Tile scheduler's reordering and the
   DRAM round trip: sum-of-squares -> sqrt -> fast-reciprocal ->
   partition-broadcast via a 1-row f32 PE matmul into PSUM -> muls,
   all emitted under tc.high_priority().
 - sims consume gather chunks CHUNK-MAJOR (both query halves of chunk
   0 before chunk 1), giving AllGather 1 ~35us of PE cover.
 - collectives + gather reads live EXCLUSIVELY on the gpsimd queue;
   sync/scalar/vector streams never wait on a collective semaphore
   (a collective-gated wait parked ahead of encoder work head-of-line
   blocks that engine and stalls the PE).
 - per-block sum-of-squares ones-matmuls trail the main groups by two
   (support) or one full block (queries), so the PE never waits on the
   ACT->DVE square chain.
 - preds for query-half 0 run between the two sims halves, so their
   divide+store tails hide under remaining sims; softmax-denominator
   reciprocals use the fast custom-DVE reciprocal (~5x faster).

Device layout: embeddings are computed TRANSPOSED ([emb, n], emb on
partitions) so the chain needs no transposes; one_hot is augmented
with a ones column so the softmax denominator falls out of the preds
matmul. Cosine sims are in [-1,1]: no max subtraction needed. Matmul
inputs are fp8e4m3 (DoubleRow, fp32 PSUM): normalized embeddings
scaled by 16, W by 32. exp stays bf16 -- fp8 exp costs ~10x in max rel
err (softmax weights concentrate; quantization noise doesn't average
out). Measured max rel err ~1.8e-3 vs the 2e-2 gate.
"""

import numpy as np
import ml_dtypes

import concourse.bacc as bacc
import concourse.mybir as mybir
import concourse.tile as tile
from concourse.bass_utils import run_bass_kernel_spmd

F32 = mybir.dt.float32
BF16 = mybir.dt.bfloat16
FP8 = mybir.dt.float8e4
EMB_SCALE = 16.0
W_SCALE = 32.0
AF = mybir.ActivationFunctionType
DR = mybir.MatmulPerfMode.DoubleRow

# Full-problem config (hardcoded; the grading harness provides exactly these)
N_SUPPORT = 4096
N_QUERY = 8192
IN_DIM = 2048
EMB_DIM = 1024
N_CLS = 64
N_CORES = 8
NQ_SHARD = N_QUERY // N_CORES  # 1024 query rows per core

# AllGather chunk widths (support rows per chunk, per core). Two chunks:
# per-AG cost is fixed-dominated (~15-23us regardless of size), so more,
# smaller chunks just stretch the serial CC pipeline; with two, chunk 0's
# chunk-major sims (~35us) fully cover AllGather 1.
CWS = (256, 256)
OFFS = (0, 256)


def build_nc(NS, NQ, IN, EMB, NCLS, n_cores=N_CORES):
    """Per-core Bass program. NS is GLOBAL support count, NQ the per-core
    query count. NCLS includes the +1 ones column."""
    KCH = IN // 128    # contraction chunks for the encoder matmul
    MCH = EMB // 128   # emb chunks (partition blocks of the embT layout)
    SCH = NS // 128    # support chunks
    NS_SH = NS // n_cores
    NB_Q = NQ // 512
    G = len(CWS)
    assert NS_SH == sum(CWS) and NQ % 512 == 0 and IN % 256 == 0
    assert all(cw % 128 == 0 for cw in CWS)

    nc = bacc.Bacc()
    # host-pre-laid-out inputs (see _prep_inputs): every DMA is contiguous
    supX = nc.declare_dram_parameter("supX", [2, 128, KCH, NS_SH // 2], FP8,
                                     isOutput=False)
    qX = nc.declare_dram_parameter("qX", [NB_Q, 128, KCH, 512], FP8,
                                   isOutput=False)
    Wd = nc.declare_dram_parameter("W", [MCH, 128, KCH, 128], FP8,
                                   isOutput=False)
    bd = nc.declare_dram_parameter("b", [128, MCH], F32, isOutput=False)
    ohd = nc.declare_dram_parameter("onehot", [128, SCH, NCLS], BF16,
                                    isOutput=False)
    outd = nc.declare_dram_parameter("out", [NQ, NCLS - 1], F32, isOutput=True)

    with tile.TileContext(nc) as tc:
        with (
            tc.tile_pool(name="singles", bufs=1) as singles,
            tc.tile_pool(name="emb_pool", bufs=1) as emb_pool,
            tc.tile_pool(name="small", bufs=2) as small,
            tc.tile_pool(name="w_pool", bufs=1) as w_pool,
            tc.tile_pool(name="xin", bufs=2) as xin,
            tc.tile_pool(name="pre_pool", bufs=2) as pre_pool,
            tc.tile_pool(name="sq_pool", bufs=2) as sq_pool,
            tc.tile_pool(name="bc_pool", bufs=2) as bc_pool,
            tc.tile_pool(name="loc_pool", bufs=1) as loc_pool,
            tc.tile_pool(name="exp_pool", bufs=1) as exp_pool,
            tc.tile_pool(name="outp", bufs=2) as outp,
            tc.tile_pool(name="dscr", bufs=2, space="DRAM") as dscr,
            tc.tile_pool(name="cc_pool", bufs=1, space="DRAM") as cc_pool,
            tc.tile_pool(name="ps_mm", bufs=3, space="PSUM") as ps_mm,
            tc.tile_pool(name="ps_n2", bufs=1, space="PSUM") as ps_n2,
            tc.tile_pool(name="ps_bc", bufs=1, space="PSUM") as ps_bc,
            tc.tile_pool(name="ps_pred", bufs=2, space="PSUM") as ps_pred,
            tc.tile_pool(name="ps_pred2", bufs=1, space="PSUM") as ps_pred2,
        ):
            ones_sb = singles.tile([128, 1], BF16)
            nc.vector.memset(ones_sb, 1.0)
            ones_row = singles.tile([1, 128], F32)
            nc.vector.memset(ones_row, 1.0)
            zeros_sb = singles.tile([128, 4 * NCLS], BF16)
            nc.vector.memset(zeros_sb, 0.0)

            # ---- inputs, first-needed first, split across queues
            b_sb = singles.tile([128, MCH], F32)
            nc.scalar.dma_start(out=b_sb, in_=bd[:, :])
            W_sb = [w_pool.tile([128, KCH, 128], FP8, tag=f"w{m}",
                                name=f"w{m}") for m in range(MCH)]
            # W[0..3] on scalar: the first support block streams on
            # sync+gpsimd in parallel, and the early W tiles no longer queue
            # behind it (v8 stalled 5.5us waiting for W2/W3 mid-encode)
            for m in range(2):
                nc.scalar.dma_start(out=W_sb[m], in_=Wd[m])
            oh_sb = singles.tile([128, SCH, NCLS], BF16)
            nc.scalar.dma_start(out=oh_sb, in_=ohd[:, :, :])
            sup_xks = []
            sA = xin.tile([128, KCH, NS_SH // 2], FP8, tag="sxk", name="sxk")
            nc.sync.dma_start(out=sA[:, 0:KCH // 2, :],
                              in_=supX[0][:, 0:KCH // 2, :])
            nc.gpsimd.dma_start(out=sA[:, KCH // 2:KCH, :],
                                in_=supX[0][:, KCH // 2:KCH, :])
            sup_xks.append(sA)
            # W[2..3] ride sync right behind the first support half: the
            # scalar queue's ACT preamble delays its triggers ~8us, which
            # left the m=2/3 groups waiting ~4us for weights
            for m in range(2, 4):
                nc.sync.dma_start(out=W_sb[m], in_=Wd[m])
            sB = xin.tile([128, KCH, NS_SH // 2], FP8, tag="sxk", name="sxk")
            nc.gpsimd.dma_start(out=sB, in_=supX[1])
            sup_xks.append(sB)
            for m in range(4, MCH):
                nc.gpsimd.dma_start(out=W_sb[m], in_=Wd[m])
            q_xks = []
            for j, eng in zip(range(NB_Q), (nc.sync, nc.scalar)):
                t = xin.tile([128, KCH, 512], FP8, tag="xk", name="xk")
                eng.dma_start(out=t, in_=qX[j])
                q_xks.append(t)

            # ---- persistent result tiles
            q_nrm = emb_pool.tile([128, MCH, NQ], FP8, name="q_nrm",
                                  tag="q_nrm")
            s_loc = loc_pool.tile([128, MCH, NS_SH], FP8, name="s_loc")
            gt = [[emb_pool.tile([128, MCH, CWS[g]], FP8, name=f"gt{g}_{c}",
                                 tag=f"gt{g}_{c}")
                   for c in range(n_cores)] for g in range(G)]
            # bf16 exp: fp8 here costs ~10x in max rel err (softmax weights
            # concentrate, so quantization noise doesn't average out)
            expT = exp_pool.tile([128, SCH, NQ], BF16)

            # ---- encode block machinery; emission is hand-interleaved so
            # every engine's instruction stream is in execution-time order
            def make_block(xk, res_tile, res_off, bs):
                st = {
                    "pre": pre_pool.tile([128, MCH, bs], BF16, tag="pre",
                                         name="pre"),
                    "sq": sq_pool.tile([128, MCH, bs], BF16, tag="sq",
                                       name="sq"),
                    "n2": None,
                }

                def group(m):
                    ps = ps_mm.tile([128, bs], F32, tag="mmps", name="ps")
                    for t in range(KCH // 2):
                        nc.tensor.matmul(
                            ps,
                            lhsT=W_sb[m][:, 2 * t:2 * t + 2, :],
                            rhs=xk[:, 2 * t:2 * t + 2, :],
                            start=(t == 0), stop=(t == KCH // 2 - 1),
                            perf_mode=DR,
                        )
                    # bias add (rescaling the fp8 W) + PSUM->SBUF bf16
                    nc.scalar.activation(st["pre"][:, m, :], ps, AF.Identity,
                                         bias=b_sb[:, m:m + 1],
                                         scale=1.0 / W_SCALE)
                    nc.vector.tensor_mul(st["sq"][:, m, :], st["pre"][:, m, :],
                                         st["pre"][:, m, :])

                def ones(m):
                    # column sums of squares via ones-matmul (partition
                    # reduce); callers sequence m = 0..MCH-1 trailing the
                    # main groups so the PE never waits on the square chain
                    if m == 0:
                        st["n2"] = ps_n2.tile([1, bs], F32, tag="n2",
                                              name="n2")
                    nc.tensor.matmul(st["n2"], lhsT=ones_sb,
                                     rhs=st["sq"][:, m, :],
                                     start=(m == 0), stop=(m == MCH - 1))

                def normalize(slices, ship_cb=None):
                    # ship path (support block): the whole chain runs at
                    # high priority AND broadcasts inv across partitions via
                    # a 1-row PE matmul into PSUM instead of the DMA round
                    # trip through DRAM -- the Tile scheduler's cost-model
                    # timeline otherwise parks the muls behind later blocks'
                    # square ops, which in v3/v4 pushed the AllGather
                    # doorbell from ~45us to ~60us (and the first AG then
                    # absorbs peer skew in-op: +20us on its duration)
                    import contextlib
                    prio = (tc.high_priority() if ship_cb is not None
                            else contextlib.nullcontext())
                    with prio:
                        nrm = small.tile([1, bs], F32, tag="nrm", name="nrm")
                        nc.scalar.activation(nrm, st["n2"], AF.Sqrt,
                                             scale=1.0 / (EMB_SCALE * EMB_SCALE))
                        inv = small.tile([1, bs], F32, tag="inv", name="inv")
                        nc.vector.reciprocal_approx_fast(inv, nrm)
                        if ship_cb is not None:
                            invb = ps_bc.tile([128, bs], F32, tag="invb",
                                              name="invb")
                            nc.tensor.matmul(invb, lhsT=ones_row, rhs=inv)
                        else:
                            # partition-broadcast via DRAM (off critical
                            # path; DMA needs a DRAM source for zero
                            # partition step)
                            iscr = dscr.tile([1, bs], F32, tag="iscr",
                                             name="iscr")
                            nc.sync.dma_start(out=iscr, in_=inv)
                            invb = bc_pool.tile([128, bs], F32, tag="invb",
                                                name="invb")
                            nc.sync.dma_start(
                                out=invb, in_=iscr.partition_broadcast(128))
                        for si, (lo, hi) in enumerate(slices):
                            for m in range(MCH):
                                nc.vector.tensor_mul(
                                    res_tile[:, m, res_off + lo:res_off + hi],
                                    st["pre"][:, m, lo:hi], invb[:, lo:hi])
                            if ship_cb is not None:
                                ship_cb(si)

                return group, ones, normalize

            ag_outs = []

            def ship(g):
                # each normalized chunk AllGathers the moment it exists
                cw = CWS[g]
                ag_in = cc_pool.tile([128, MCH * cw], FP8, name=f"ag_in{g}",
                                     tag=f"ag_in{g}")
                nc.sync.dma_start(
                    out=ag_in.rearrange("p (m v) -> p m v", m=MCH),
                    in_=s_loc[:, :, OFFS[g]:OFFS[g] + cw])
                ag_out = cc_pool.tile(
                    [n_cores * 128, MCH * cw], FP8, name=f"ag_out{g}",
                    tag=f"ag_out{g}", addr_space="Shared")
                nc.gpsimd.collective_compute(
                    "AllGather", mybir.AluOpType.bypass,
                    replica_groups=[list(range(n_cores))],
                    ins=[ag_in], outs=[ag_out],
                )
                ag_outs.append(ag_out)

            # support runs as TWO 256-row blocks: chunk 0's full
            # normalize+ship chain completes ~20us earlier than with one
            # 512-row block, so the AllGather doorbell beats the CC-stream
            # barrier and peer skew is absorbed before the op (not in it).
            # The N=256 LDWEIGHTS-bound penalty (~6us PE) is free: the
            # encode tail hides under the collective pipeline anyway.
            SAg, SAones, SAnorm = make_block(sup_xks[0], s_loc, 0, 256)
            SBg, SBones, SBnorm = make_block(sup_xks[1], s_loc, 256, 256)
            Q0g, Q0ones, Q0norm = make_block(q_xks[0], q_nrm, 0, 512)
            Q1g, Q1ones, Q1norm = make_block(q_xks[1], q_nrm, 512, 512)

            for m in range(MCH):
                SAg(m)
                if m >= 2:
                    SAones(m - 2)
            SBg(0)
            SAones(MCH - 2)
            SAones(MCH - 1)
            SBg(1)
            SAnorm([(0, 256)], ship_cb=lambda si: ship(0))
            for m in range(2, MCH):
                SBg(m)
                if m >= 4:
                    SBones(m - 4)
            Q0g(0)
            for m in range(MCH - 4, MCH):
                SBones(m)
            Q0g(1)
            SBnorm([(0, 256)], ship_cb=lambda si: ship(1))
            for m in range(2, MCH):
                Q0g(m)
            Q1g(0)
            for m in range(MCH):
                Q0ones(m)
            Q1g(1)
            Q0norm([(0, 512)])
            for m in range(2, MCH):
                Q1g(m)
                if m >= 4:
                    Q1ones(m - 4)

            # gather-read DMAs: gpsimd queue ONLY (they wait on collective
            # semaphores; nothing else may queue behind them)
            for g in range(G):
                for c in range(n_cores):
                    nc.gpsimd.dma_start(
                        out=gt[g][c],
                        in_=ag_outs[g][c * 128:(c + 1) * 128, :]
                            .rearrange("p (m v) -> p m v", m=MCH),
                    )

            # ---- sims + softmax-exp, [sup, q] layout, chunk-arrival order
            work = [(c * (NS_SH // 128) + OFFS[g] // 128 + i, gt[g][c], i)
                    for g in range(G)
                    for c in range(n_cores)
                    for i in range(CWS[g] // 128)]
            assert MCH % 2 == 0 and len(work) == SCH

            def sims_tile(sb, src, i, qh):
                qs = slice(qh * 512, (qh + 1) * 512)
                ps = ps_mm.tile([128, 512], F32, tag="mmps", name="ps")
                for t in range(MCH // 2):
                    nc.tensor.matmul(
                        ps,
                        lhsT=src[:, 2 * t:2 * t + 2, i * 128:(i + 1) * 128],
                        rhs=q_nrm[:, 2 * t:2 * t + 2, qs],
                        start=(t == 0), stop=(t == MCH // 2 - 1),
                        perf_mode=DR,
                    )
                nc.scalar.activation(expT[:, sb, qs], ps, AF.Exp,
                                     scale=1.0 / (EMB_SCALE * EMB_SCALE))

            # support-chunk index sets for split preds accumulation
            CH0 = sorted(c * (NS_SH // 128) + i for c in range(n_cores)
                         for i in range(CWS[0] // 128))
            CH1 = [sb for sb in range(SCH) if sb not in set(CH0)]
            pp_open = {}

            def preds_div(qb, pp):
                # softmax denominator is the ones column; divide
                qs = slice(qb * 128, (qb + 1) * 128)
                rec = small.tile([128, 1], F32, tag="rec", name="rec")
                nc.vector.reciprocal_approx_fast(rec, pp[:, NCLS - 1:NCLS])
                ot = outp.tile([128, NCLS - 1], F32, tag="ot", name="ot")
                nc.vector.tensor_scalar_mul(ot, pp[:, 0:NCLS - 1], rec)
                nc.sync.dma_start(out=outd[qs, :], in_=ot)

            def preds(qb):
                qs = slice(qb * 128, (qb + 1) * 128)
                pp = ps_pred.tile([128, NCLS], F32, tag="pp", name="pp")
                for sb in range(SCH):
                    nc.tensor.matmul(pp, lhsT=expT[:, sb, qs],
                                     rhs=oh_sb[:, sb, :],
                                     start=(sb == 0), stop=(sb == SCH - 1))
                preds_div(qb, pp)

            def preds_start(qb):
                # accumulate the chunk-0 half mid-sims; group stays open.
                # All four open accumulators share one PSUM bank (pools are
                # bank-granular; 4 x 65 fp32 = 1040B fits one 2KB bank).
                # start=True clears the whole BANK, so the bank is zeroed
                # ONCE by a dummy matmul against a zero rhs, and every real
                # accumulation uses start=False.
                qs = slice(qb * 128, (qb + 1) * 128)
                if "t" not in pp_open:
                    t = ps_pred2.tile([128, 4, NCLS], F32, tag="pp2",
                                      name="pp2")
                    pp_open["t"] = t
                    nc.tensor.matmul(
                        t.rearrange("p a c -> p (a c)"),
                        lhsT=W_sb[0][:, 0:1, :], rhs=zeros_sb,
                        start=True, stop=False, skip_group_check=True)
                pp = pp_open["t"][:, qb - NQ // 256, :]
                pp_open[qb] = pp
                for k, sb in enumerate(CH0):
                    nc.tensor.matmul(pp, lhsT=expT[:, sb, qs],
                                     rhs=oh_sb[:, sb, :],
                                     start=False, stop=False,
                                     skip_group_check=True)

            def preds_finish(qb):
                qs = slice(qb * 128, (qb + 1) * 128)
                pp = pp_open[qb]
                for k, sb in enumerate(CH1):
                    nc.tensor.matmul(pp, lhsT=expT[:, sb, qs],
                                     rhs=oh_sb[:, sb, :],
                                     start=False, stop=(k == len(CH1) - 1),
                                     skip_group_check=True)
                preds_div(qb, pp)

            # chunk-major sims: BOTH query halves of chunk 0 run before
            # chunk 1 is touched, so AllGather 1 gets ~35us of PE cover
            # after AllGather 0 lands (CC timing varies 2x run to run).
            # Q1's deferred ones + normalize slot in behind the first two
            # tiles (their PE cover absorbs the square-chain lag).
            ws, pos = [], 0
            for g in range(G):
                n_g = n_cores * (CWS[g] // 128)
                ws.append(work[pos:pos + n_g])
                pos += n_g
            # Q1's last ones + normalize go BEFORE the first sims tile:
            # sims tile 0 blocks on AllGather 0, and this ready work would
            # otherwise sit parked behind it during the stall (ones m0..m3
            # already trailed inside Q1's groups, mirroring SA/SB)
            for m in range(MCH - 4, MCH):
                Q1ones(m)
            Q1norm([(0, 512)])
            for wk in ws[0]:
                sims_tile(*wk, 0)
            for wk in ws[0]:
                sims_tile(*wk, 1)
            wl = ws[G - 1]
            # chunk-0 halves of the qh=1 preds accumulate mid-sims (two
            # chunk-1 tiles first cover the exp tail of chunk 0's qh=1),
            # halving the preds work left after the final sims tile
            sims_tile(*wl[0], 0)
            sims_tile(*wl[1], 0)
            for qb in range(NQ // 256, NQ // 128):
                preds_start(qb)
            for wk in wl[2:]:
                sims_tile(*wk, 0)
            # preds for query-half 0 slot in after two qh=1 tiles so their
            # divide+store tails hide under the remaining sims
            sims_tile(*wl[0], 1)
            sims_tile(*wl[1], 1)
            for qb in range(NQ // 256):
                preds(qb)
            for wk in wl[2:]:
                sims_tile(*wk, 1)
            for qb in range(NQ // 256, NQ // 128):
                preds_finish(qb)

    nc.finalize()
    return nc


_NC_CACHE = {}


def _get_nc(key):
    if key not in _NC_CACHE:
        NS, NQ, IN, EMB, NCLS = key
        _NC_CACHE[key] = build_nc(NS, NQ, IN, EMB, NCLS)
    return _NC_CACHE[key]


def _x_layout(x, kch, bs=512):
    """[NV, IN] fp32 -> [NV/bs, 128, KCH, bs] fp8 so each bs-row encoder
    block is one contiguous DMA: H[nb,p,k,v] = x[nb*bs+v, k*128+p]."""
    nv, in_dim = x.shape
    h = x.reshape(nv // bs, bs, kch, 128).transpose(0, 3, 2, 1)
    return np.ascontiguousarray(h.astype(ml_dtypes.float8_e4m3))


def _prep_inputs(support, query, W, b, support_labels, num_classes, n_cores):
    ncls = int(num_classes)
    bf = ml_dtypes.bfloat16
    support = np.asarray(support, np.float32)
    query = np.asarray(query, np.float32)
    W = np.asarray(W, np.float32)
    in_dim, emb = W.shape
    kch, mch = in_dim // 128, emb // 128
    ns = support.shape[0]
    # W[m, p, k, e] = W_SCALE * W[k*128+p, m*128+e]
    Wh = np.ascontiguousarray(
        (W * W_SCALE).reshape(kch, 128, mch, 128)
        .transpose(2, 1, 0, 3).astype(ml_dtypes.float8_e4m3))
    # b[p, m] = b[m*128+p]
    bh = np.ascontiguousarray(np.asarray(b, np.float32).reshape(mch, 128).T)
    labels = np.asarray(support_labels).astype(np.int64)
    oh = np.zeros((ns, ncls + 1), dtype=bf)
    oh[np.arange(ns), labels] = 1
    oh[:, ncls] = 1  # ones column -> softmax denominator
    # oh[p, c, h] = onehot[c*128+p, h]
    ohh = np.ascontiguousarray(
        oh.reshape(ns // 128, 128, ncls + 1).transpose(1, 0, 2))
    nq_shard = query.shape[0] // n_cores
    ns_shard = ns // n_cores
    qh_all = _x_layout(query, kch)  # [NQ/512, 128, KCH, 512]
    nbq = nq_shard // 512
    in_maps = []
    for i in range(n_cores):
        supx = _x_layout(support[i * ns_shard:(i + 1) * ns_shard], kch,
                         ns_shard // 2)  # [2, 128, KCH, NS_SH/2]
        in_maps.append({
            "supX": supx,
            "qX": np.ascontiguousarray(qh_all[i * nbq:(i + 1) * nbq]),
            "W": Wh,
            "b": bh,
            "onehot": ohh,
        })
    return in_maps


def _run(support, query, W, b, support_labels, num_classes, trace=False):
    ncls = int(num_classes)
    key = (support.shape[0], query.shape[0] // N_CORES, support.shape[1],
           W.shape[1], ncls + 1)
    nc = _get_nc(key)
    in_maps = _prep_inputs(support, query, W, b, support_labels, ncls, N_CORES)
    res = run_bass_kernel_spmd(nc, in_maps, list(range(N_CORES)), trace=trace)
    out = np.concatenate([r["out"] for r in res.results], axis=0)
    return out.astype(np.float32), res


def kernel(support, query, W, b, support_labels, num_classes):
    out, _ = _run(support, query, W, b, support_labels, num_classes, trace=False)
    return out
